# revision 13
# baseline (speedup 1.0000x reference)
"""Channel-attention (CTrans2) Trainium2 kernel.

Math per batch d (D=8, C=512, HW=4096):
    q = x.reshape(C, HW)
    energy = q @ q.T                        # (C, C)
    en = rowmax(energy) - energy
    a1 = softmax(en, axis=-1)
    a2 = softmax(a1 + atten, axis=-1)
    out = a2.T @ q                          # (C, HW)
    result = gamma * out + x

Sharding: data-parallel over D, one batch per NeuronCore (8 cores).

Implementation notes:
  - Matmuls run in float32r (fp32 storage, ~11-bit-mantissa multiply, fp32
    accumulate) at full PE rate. q arrives via DMA from f32r-declared DRAM;
    qT is produced on-chip with PE transposes interleaved with the first
    matmul so the PE chases the input DMA stream.
  - Softmax matches the reference's fp32 op order: en = (-energy) + rowmax,
    m = rowmax(en), exp(en - m) / sum.
"""

import numpy as np

import concourse.bacc as bacc
import concourse.tile as tile
import concourse.mybir as mybir
from concourse import masks
from concourse import bass_utils

D, C, HW = 8, 512, 4096
P = 128          # partitions
NB = HW // 512   # 8 column blocks of q
CB = C // P      # 4 channel blocks
KT = HW // P     # 32 transposed k-chunks
LAG = 3          # transpose-ahead distance before mm1 consumes a k-chunk

F32 = mybir.dt.float32
F32R = mybir.dt.float32r
AF = mybir.ActivationFunctionType
ALU = mybir.AluOpType
AX = mybir.AxisListType

COPY_ENGINE = "scalar"   # engine for psum->sbuf qT copies ("scalar" or "vector")

_CACHE = {}


def _build(loop_t=None):
    nc = bacc.Bacc("TRN2", target_bir_lowering=False, debug=False)

    x_d = nc.dram_tensor("x", [C, HW], F32R, kind="ExternalInput").ap()
    at_d = nc.dram_tensor("atten", [C, C], F32, kind="ExternalInput").ap()
    g_d = nc.dram_tensor("gamma_b", [P, 1], F32, kind="ExternalInput").ap()
    out_d = nc.dram_tensor("out", [C, HW], F32, kind="ExternalOutput").ap()

    with tile.TileContext(nc) as tc:
        from contextlib import ExitStack
        with (
            tc.tile_pool(name="qp", bufs=1) as qp,
            tc.tile_pool(name="qtp", bufs=1) as qtp,
            tc.tile_pool(name="smax", bufs=2) as smax,
            tc.tile_pool(name="small", bufs=1) as small,
            tc.tile_pool(name="outp", bufs=2) as outp,
            tc.tile_pool(name="ps_sh", bufs=4, space="PSUM") as ps_sh,
            tc.tile_pool(name="ps_e", bufs=1, space="PSUM") as ps_e,
            ExitStack() as loop_ctx,
        ):
            # ---- constants
            gam = small.tile([P, 1], F32, tag="gam")
            nc.sync.dma_start(gam[:], g_d[:])
            ident = small.tile([P, P], F32, tag="ident")
            masks.make_identity(nc, ident[:])
            identr = small.tile([P, P], F32R, tag="identr")
            nc.vector.tensor_copy(identr[:], ident[:])

            att_in = [small.tile([P, C], F32, tag=f"attin{ci}", name=f"attin{ci}")
                      for ci in range(CB)]

            if loop_t is not None:
                loop_ctx.enter_context(tc.For_i(0, loop_t, 1))

            # ---- load q: q[ci][h] is (128, 2048) f32r, DMA'd in (128, 512)
            # chunks ordered nb-major so transpose k-groups unlock early
            q = [[qp.tile([P, 2048], F32R, tag=f"q{ci}_{h}", name=f"q{ci}_{h}")
                  for h in range(2)] for ci in range(CB)]
            for nb in range(NB):
                h, r = divmod(nb, 4)
                for ci in range(CB):
                    nc.sync.dma_start(
                        q[ci][h][:, r * 512:(r + 1) * 512],
                        x_d[ci * P:(ci + 1) * P, nb * 512:(nb + 1) * 512])
            for ci in range(CB):
                nc.sync.dma_start(att_in[ci][:], at_d[ci * P:(ci + 1) * P, :])

            def q_slice(ci, nb):
                """(128, 512) f32r view of channel-block ci, n-block nb."""
                h, r = divmod(nb, 4)
                return q[ci][h][:, r * 512:(r + 1) * 512]

            def q_kslice(ci, k):
                """(128, 128) f32r view for transpose chunk k."""
                h, r = divmod(k, 16)
                return q[ci][h][:, r * P:(r + 1) * P]

            # ---- interleaved: PE transposes (k) + mm1 matmuls (k - LAG)
            qt = [qtp.tile([P, C], F32R, tag=f"qt{k}", name=f"qt{k}")
                  for k in range(KT)]
            pe_ = [ps_e.tile([P, C], F32, tag=f"e{mi}", name=f"pe{mi}")
                   for mi in range(CB)]
            copy_eng = getattr(nc, COPY_ENGINE)

            def emit_transpose(k):
                ptr = ps_sh.tile([P, C], F32R, tag="sh", name=f"tr{k}")
                for ci in range(CB):
                    nc.tensor.transpose(
                        ptr[:, ci * P:(ci + 1) * P], q_kslice(ci, k), identr[:])
                if COPY_ENGINE == "scalar":
                    copy_eng.copy(qt[k][:], ptr[:])
                else:
                    copy_eng.tensor_copy(qt[k][:], ptr[:])

            def emit_mm1(k, mis):
                for mi in mis:
                    nc.tensor.matmul(
                        pe_[mi][:], qt[k][:, mi * P:(mi + 1) * P], qt[k][:],
                        start=(k == 0), stop=(k == KT - 1))

            # interleave transposes with mm1 for k < KT-TAIL; then stagger
            # the last TAIL k's per mi so the softmax latency chain for
            # mi=0 hides under the mm1 tails of mi=1..3
            TAIL = 16
            for k in range(KT + LAG):
                if k < KT:
                    emit_transpose(k)
                if k >= LAG and k - LAG < KT - TAIL:
                    emit_mm1(k - LAG, range(CB))
            for mi in range(CB):
                for k in range(KT - TAIL, KT):
                    emit_mm1(k, [mi])

            # ---- softmax chain per channel-block mi
            a2 = [smax.tile([P, C], F32R, tag=f"a2_{mi}", name=f"a2_{mi}")
                  for mi in range(CB)]
            for mi in range(CB):
                pe = pe_[mi]
                rmax = smax.tile([P, 1], F32, tag="rmax")
                nc.vector.tensor_reduce(rmax[:], pe[:], axis=AX.X, op=ALU.max)
                en = smax.tile([P, C], F32, tag="en")
                nc.vector.tensor_scalar(
                    en[:], pe[:], -1.0, rmax[:], op0=ALU.mult, op1=ALU.add)
                nm = smax.tile([P, 1], F32, tag="nm")
                nc.vector.tensor_reduce(
                    nm[:], en[:], axis=AX.X, op=ALU.max, negate=True)
                e1 = smax.tile([P, C], F32, tag="e1")
                s1 = smax.tile([P, 1], F32, tag="s1")
                nc.scalar.activation(
                    e1[:], en[:], AF.Exp, bias=nm[:], scale=1.0, accum_out=s1[:])
                r1 = smax.tile([P, 1], F32, tag="r1")
                nc.vector.reciprocal(r1[:], s1[:])

                z = smax.tile([P, C], F32, tag="z")
                nc.vector.scalar_tensor_tensor(
                    z[:], e1[:], r1[:], att_in[mi][:], op0=ALU.mult, op1=ALU.add)
                nm2 = smax.tile([P, 1], F32, tag="nm2")
                nc.vector.tensor_reduce(
                    nm2[:], z[:], axis=AX.X, op=ALU.max, negate=True)
                e2 = smax.tile([P, C], F32, tag="e2")
                s2 = smax.tile([P, 1], F32, tag="s2")
                nc.scalar.activation(
                    e2[:], z[:], AF.Exp, bias=nm2[:], scale=1.0, accum_out=s2[:])
                r2 = smax.tile([P, 1], F32, tag="r2")
                nc.vector.reciprocal(r2[:], s2[:])
                nc.vector.tensor_scalar_mul(a2[mi][:], e2[:], r2[:])

            # ---- mm2 + epilogue; out staged per (mj, half) -> 8 x 1MB DMAs
            # epilogue alternates between a single DVE op and an ACT-mul +
            # DVE-add pair to balance the two engines in this phase
            for mj in range(CB):
                for h in range(2):
                    ot = outp.tile([P, 2048], F32, tag="ot", name=f"ot{mj}_{h}")
                    for r in range(4):
                        nb = h * 4 + r
                        po = ps_sh.tile([P, 512], F32, tag="sh", name=f"po{mj}_{nb}")
                        for ki in range(CB):
                            nc.tensor.matmul(
                                po[:], a2[ki][:, mj * P:(mj + 1) * P],
                                q_slice(ki, nb),
                                start=(ki == 0), stop=(ki == CB - 1))
                        osl = ot[:, r * 512:(r + 1) * 512]
                        if nb % 2 == 0:
                            nc.vector.scalar_tensor_tensor(
                                osl, po[:], gam[:],
                                q_slice(mj, nb).bitcast(F32),
                                op0=ALU.mult, op1=ALU.add)
                        else:
                            go = smax.tile([P, 512], F32, tag="go",
                                           name=f"go{mj}_{nb}")
                            nc.scalar.mul(go[:], po[:], gam[:])
                            nc.vector.tensor_add(
                                osl, go[:], q_slice(mj, nb).bitcast(F32))
                    nc.sync.dma_start(
                        out_d[mj * P:(mj + 1) * P, h * 2048:(h + 1) * 2048],
                        ot[:])

    nc.compile()
    return nc


def get_nc(loop_t=None):
    key = ("nc", loop_t)
    if key not in _CACHE:
        _CACHE[key] = _build(loop_t)
    return _CACHE[key]


def make_in_maps(inputs):
    x, atten, gamma = inputs["x"], inputs["atten"], inputs["gamma"]
    gb = np.broadcast_to(np.asarray(gamma, np.float32).reshape(1, 1), (P, 1)).copy()
    return [
        {
            "x": np.ascontiguousarray(np.asarray(x[d], np.float32).reshape(C, HW)),
            "atten": np.ascontiguousarray(np.asarray(atten[d], np.float32)),
            "gamma_b": gb,
        }
        for d in range(D)
    ]


def kernel(x: np.ndarray, atten: np.ndarray, gamma: np.ndarray) -> np.ndarray:
    assert x.shape == (D, C, 64, 64) and atten.shape == (D, C, C)
    nc = get_nc()
    in_maps = make_in_maps({"x": x, "atten": atten, "gamma": gamma})
    res = bass_utils.run_bass_kernel_spmd(nc, in_maps, list(range(D)))
    out = np.stack([res.results[d]["out"] for d in range(D)])
    return out.reshape(D, C, 64, 64).astype(np.float32)


# revision 16
# speedup vs baseline: 1.4468x; 1.4468x over previous
"""Channel-attention (CTrans2) Trainium2 kernel.

Math per batch d (D=8, C=512, HW=4096):
    q = x.reshape(C, HW)
    energy = q @ q.T                        # (C, C)
    en = rowmax(energy) - energy
    a1 = softmax(en, axis=-1)
    a2 = softmax(a1 + atten, axis=-1)
    out = a2.T @ q                          # (C, HW)
    result = gamma * out + x

Sharding: data-parallel over D, one batch per NeuronCore (8 cores).

Implementation notes:
  - Matmuls run in float32r (fp32 storage, ~11-bit-mantissa multiply, fp32
    accumulate) at full PE rate. q arrives via DMA from f32r-declared DRAM;
    qT is produced on-chip with PE transposes interleaved with the first
    matmul so the PE chases the input DMA stream.
  - Softmax matches the reference's fp32 op order: en = (-energy) + rowmax,
    m = rowmax(en), exp(en - m) / sum.
"""

import numpy as np

import concourse.bacc as bacc
import concourse.tile as tile
import concourse.mybir as mybir
from concourse import masks
from concourse import bass_utils

D, C, HW = 8, 512, 4096
P = 128          # partitions
NB = HW // 512   # 8 column blocks of q
CB = C // P      # 4 channel blocks
KT = HW // P     # 32 transposed k-chunks
LAG = 3          # transpose-ahead distance before mm1 consumes a k-chunk

F32 = mybir.dt.float32
F32R = mybir.dt.float32r
AF = mybir.ActivationFunctionType
ALU = mybir.AluOpType
AX = mybir.AxisListType

COPY_ENGINE = "scalar"   # engine for psum->sbuf qT copies ("scalar" or "vector")

_CACHE = {}


def _build(loop_t=None):
    nc = bacc.Bacc("TRN2", target_bir_lowering=False, debug=False)

    x_d = nc.dram_tensor("x", [C, HW], F32R, kind="ExternalInput").ap()
    at_d = nc.dram_tensor("atten", [C, C], F32, kind="ExternalInput").ap()
    g_d = nc.dram_tensor("gamma_b", [P, 1], F32, kind="ExternalInput").ap()
    out_d = nc.dram_tensor("out", [C, HW], F32, kind="ExternalOutput").ap()

    with tile.TileContext(nc) as tc:
        from contextlib import ExitStack
        with (
            tc.tile_pool(name="qp", bufs=1) as qp,
            tc.tile_pool(name="qtp", bufs=1) as qtp,
            tc.tile_pool(name="smax", bufs=2) as smax,
            tc.tile_pool(name="small", bufs=1) as small,
            tc.tile_pool(name="outp", bufs=2) as outp,
            tc.tile_pool(name="ps_sh", bufs=4, space="PSUM") as ps_sh,
            tc.tile_pool(name="ps_e", bufs=1, space="PSUM") as ps_e,
            ExitStack() as loop_ctx,
        ):
            # ---- constants
            gam = small.tile([P, 1], F32, tag="gam")
            nc.sync.dma_start(gam[:], g_d[:])
            ident = small.tile([P, P], F32, tag="ident")
            masks.make_identity(nc, ident[:])
            identr = small.tile([P, P], F32R, tag="identr")
            nc.vector.tensor_copy(identr[:], ident[:])

            att_in = [small.tile([P, C], F32, tag=f"attin{ci}", name=f"attin{ci}")
                      for ci in range(CB)]

            if loop_t is not None:
                loop_ctx.enter_context(tc.For_i(0, loop_t, 1))

            # ---- load q: q[ci][h] is (128, 2048) f32r, DMA'd in (128, 512)
            # chunks ordered nb-major so transpose k-groups unlock early
            q = [[qp.tile([P, 2048], F32R, tag=f"q{ci}_{h}", name=f"q{ci}_{h}")
                  for h in range(2)] for ci in range(CB)]
            for nb in range(NB):
                h, r = divmod(nb, 4)
                for ci in range(CB):
                    nc.sync.dma_start(
                        q[ci][h][:, r * 512:(r + 1) * 512],
                        x_d[ci * P:(ci + 1) * P, nb * 512:(nb + 1) * 512])
            for ci in range(CB):
                nc.sync.dma_start(att_in[ci][:], at_d[ci * P:(ci + 1) * P, :])

            def q_slice(ci, nb):
                """(128, 512) f32r view of channel-block ci, n-block nb."""
                h, r = divmod(nb, 4)
                return q[ci][h][:, r * 512:(r + 1) * 512]

            def q_kslice(ci, k):
                """(128, 128) f32r view for transpose chunk k."""
                h, r = divmod(k, 16)
                return q[ci][h][:, r * P:(r + 1) * P]

            # ---- interleaved: PE transposes (k) + mm1 matmuls (k - LAG)
            qt = [qtp.tile([P, C], F32R, tag=f"qt{k}", name=f"qt{k}")
                  for k in range(KT)]
            pe_ = [ps_e.tile([P, C], F32, tag=f"e{mi}", name=f"pe{mi}")
                   for mi in range(CB)]
            copy_eng = getattr(nc, COPY_ENGINE)

            def emit_transpose(k):
                ptr = ps_sh.tile([P, C], F32R, tag="sh", name=f"tr{k}")
                for ci in range(CB):
                    nc.tensor.transpose(
                        ptr[:, ci * P:(ci + 1) * P], q_kslice(ci, k), identr[:])
                if COPY_ENGINE == "scalar":
                    copy_eng.copy(qt[k][:], ptr[:])
                else:
                    copy_eng.tensor_copy(qt[k][:], ptr[:])

            def emit_mm1(k, mis):
                for mi in mis:
                    nc.tensor.matmul(
                        pe_[mi][:], qt[k][:, mi * P:(mi + 1) * P], qt[k][:],
                        start=(k == 0), stop=(k == KT - 1))

            # interleave transposes with mm1 for k < KT-TAIL; then stagger
            # the last TAIL k's per mi so the softmax latency chain for
            # mi=0 hides under the mm1 tails of mi=1..3
            TAIL = 16
            for k in range(KT + LAG):
                if k < KT:
                    emit_transpose(k)
                if k >= LAG and k - LAG < KT - TAIL:
                    emit_mm1(k - LAG, range(CB))
            for mi in range(CB):
                for k in range(KT - TAIL, KT):
                    emit_mm1(k, [mi])

            # ---- softmax chain per channel-block mi
            a2 = [smax.tile([P, C], F32R, tag=f"a2_{mi}", name=f"a2_{mi}")
                  for mi in range(CB)]
            for mi in range(CB):
                pe = pe_[mi]
                # softmax(rowmax - energy) == exp(rowmin - energy) / sum
                rmin = smax.tile([P, 1], F32, tag="rmin")
                nc.vector.tensor_reduce(rmin[:], pe[:], axis=AX.X, op=ALU.min)
                e1 = smax.tile([P, C], F32, tag="e1")
                s1 = smax.tile([P, 1], F32, tag="s1")
                nc.scalar.activation(
                    e1[:], pe[:], AF.Exp, bias=rmin[:], scale=-1.0,
                    accum_out=s1[:])
                r1 = smax.tile([P, 1], F32, tag="r1")
                nc.vector.reciprocal(r1[:], s1[:])

                z = smax.tile([P, C], F32, tag="z")
                nc.vector.scalar_tensor_tensor(
                    z[:], e1[:], r1[:], att_in[mi][:], op0=ALU.mult, op1=ALU.add)
                nm2 = smax.tile([P, 1], F32, tag="nm2")
                nc.vector.tensor_reduce(
                    nm2[:], z[:], axis=AX.X, op=ALU.max, negate=True)
                e2 = smax.tile([P, C], F32, tag="e2")
                s2 = smax.tile([P, 1], F32, tag="s2")
                nc.scalar.activation(
                    e2[:], z[:], AF.Exp, bias=nm2[:], scale=1.0, accum_out=s2[:])
                r2 = smax.tile([P, 1], F32, tag="r2")
                nc.vector.reciprocal(r2[:], s2[:])
                nc.vector.tensor_scalar_mul(a2[mi][:], e2[:], r2[:])

            # ---- mm2 + epilogue; out staged per (mj, half) -> 8 x 1MB DMAs
            # epilogue alternates between a single DVE op and an ACT-mul +
            # DVE-add pair to balance the two engines in this phase
            for mj in range(CB):
                for h in range(2):
                    ot = outp.tile([P, 2048], F32, tag="ot", name=f"ot{mj}_{h}")
                    for r in range(4):
                        nb = h * 4 + r
                        po = ps_sh.tile([P, 512], F32, tag="sh", name=f"po{mj}_{nb}")
                        for ki in range(CB):
                            nc.tensor.matmul(
                                po[:], a2[ki][:, mj * P:(mj + 1) * P],
                                q_slice(ki, nb),
                                start=(ki == 0), stop=(ki == CB - 1))
                        osl = ot[:, r * 512:(r + 1) * 512]
                        if nb % 2 == 0:
                            nc.vector.scalar_tensor_tensor(
                                osl, po[:], gam[:],
                                q_slice(mj, nb).bitcast(F32),
                                op0=ALU.mult, op1=ALU.add)
                        else:
                            go = smax.tile([P, 512], F32, tag="go",
                                           name=f"go{mj}_{nb}")
                            nc.scalar.mul(go[:], po[:], gam[:])
                            nc.vector.tensor_add(
                                osl, go[:], q_slice(mj, nb).bitcast(F32))
                        if nb % 2 == 1:
                            nc.sync.dma_start(
                                out_d[mj * P:(mj + 1) * P,
                                      (nb - 1) * 512:(nb + 1) * 512],
                                ot[:, (r - 1) * 512:(r + 1) * 512])

    nc.compile()
    return nc


def get_nc(loop_t=None):
    key = ("nc", loop_t)
    if key not in _CACHE:
        _CACHE[key] = _build(loop_t)
    return _CACHE[key]


def make_in_maps(inputs):
    x, atten, gamma = inputs["x"], inputs["atten"], inputs["gamma"]
    gb = np.broadcast_to(np.asarray(gamma, np.float32).reshape(1, 1), (P, 1)).copy()
    return [
        {
            "x": np.ascontiguousarray(np.asarray(x[d], np.float32).reshape(C, HW)),
            "atten": np.ascontiguousarray(np.asarray(atten[d], np.float32)),
            "gamma_b": gb,
        }
        for d in range(D)
    ]


class _Executor:
    """Prebuilt sharded PJRT executable for repeat kernel() calls.

    Mirrors bass2jax.run_bass_via_pjrt's multi-core path, but the jitted
    program is built once and reused, so repeat calls only pay transfers.
    """

    def __init__(self, nc):
        import jax
        import jax.numpy as jnp
        from jax.experimental.shard_map import shard_map
        from jax.sharding import Mesh, PartitionSpec, NamedSharding
        from concourse import bass2jax

        bass2jax.install_neuronx_cc_hook()
        self._jax = jax
        pname = nc.partition_id_tensor.name if nc.partition_id_tensor else None
        in_names, out_names, out_avals = [], [], []
        for alloc in nc.m.functions[0].allocations:
            if not isinstance(alloc, mybir.MemoryLocationSet):
                continue
            name = alloc.memorylocations[0].name
            if alloc.kind == "ExternalInput":
                if name != pname:
                    in_names.append(name)
            elif alloc.kind == "ExternalOutput":
                out_names.append(name)
                out_avals.append(jax.core.ShapedArray(
                    tuple(alloc.tensor_shape), mybir.dt.np(alloc.dtype)))
        self.in_names, self.out_names, self.out_avals = \
            in_names, out_names, out_avals
        all_in = in_names + out_names + ([pname] if pname else [])

        def _body(*args):
            operands = list(args)
            if pname is not None:
                operands.append(bass2jax.partition_id_tensor())
            return tuple(bass2jax._bass_exec_p.bind(
                *operands,
                out_avals=tuple(out_avals),
                in_names=tuple(all_in),
                out_names=tuple(out_names),
                lowering_input_output_aliases=(),
                sim_require_finite=True,
                sim_require_nnan=True,
                nc=nc,
            ))

        devices = jax.devices()[:D]
        mesh = Mesh(np.asarray(devices), ("core",))
        n_params, n_outs = len(in_names), len(out_names)
        self.sharding = NamedSharding(mesh, PartitionSpec("core"))
        self.sharded = jax.jit(
            shard_map(
                _body, mesh=mesh,
                in_specs=(PartitionSpec("core"),) * (n_params + n_outs),
                out_specs=(PartitionSpec("core"),) * n_outs,
                check_rep=False,
            ),
            donate_argnums=tuple(range(n_params, n_params + n_outs)),
            keep_unused=True,
        )
        zshapes = [(D * a.shape[0], *a.shape[1:]) for a in out_avals]
        zdtypes = [a.dtype for a in out_avals]
        self.mk_zeros = jax.jit(
            lambda: tuple(jnp.zeros(s, d) for s, d in zip(zshapes, zdtypes)),
            out_shardings=tuple(self.sharding for _ in zshapes),
        )

    def run(self, in_maps):
        jax = self._jax
        din = [
            jax.device_put(
                np.concatenate([np.asarray(m[name]) for m in in_maps], axis=0),
                self.sharding)
            for name in self.in_names
        ]
        outs = self.sharded(*din, *self.mk_zeros())
        return [
            {name: np.asarray(outs[i]).reshape(D, *self.out_avals[i].shape)[d]
             for i, name in enumerate(self.out_names)}
            for d in range(D)
        ]


def _get_executor():
    if "exec" not in _CACHE:
        _CACHE["exec"] = _Executor(get_nc())
    return _CACHE["exec"]


def kernel(x: np.ndarray, atten: np.ndarray, gamma: np.ndarray) -> np.ndarray:
    assert x.shape == (D, C, 64, 64) and atten.shape == (D, C, C)
    in_maps = make_in_maps({"x": x, "atten": atten, "gamma": gamma})
    try:
        results = _get_executor().run(in_maps)
    except Exception:
        res = bass_utils.run_bass_kernel_spmd(get_nc(), in_maps, list(range(D)))
        results = res.results
    out = np.stack([results[d]["out"] for d in range(D)])
    return out.reshape(D, C, 64, 64).astype(np.float32)


# revision 20
# speedup vs baseline: 13.3992x; 9.2610x over previous
"""Channel-attention (CTrans2) Trainium2 kernel.

Math per batch d (D=8, C=512, HW=4096):
    q = x.reshape(C, HW)
    energy = q @ q.T                        # (C, C)
    en = rowmax(energy) - energy
    a1 = softmax(en, axis=-1)
    a2 = softmax(a1 + atten, axis=-1)
    out = a2.T @ q                          # (C, HW)
    result = gamma * out + x

Sharding: data-parallel over D, one batch per NeuronCore (8 cores).

Implementation notes:
  - Matmuls run in float32r (fp32 storage, ~11-bit-mantissa multiply, fp32
    accumulate) at full PE rate. q arrives via DMA from f32r-declared DRAM;
    qT is produced on-chip with PE transposes interleaved with the first
    matmul so the PE chases the input DMA stream.
  - Softmax matches the reference's fp32 op order: en = (-energy) + rowmax,
    m = rowmax(en), exp(en - m) / sum.
"""

import numpy as np

import concourse.bacc as bacc
import concourse.tile as tile
import concourse.mybir as mybir
from concourse import masks
from concourse import bass_utils

D, C, HW = 8, 512, 4096
P = 128          # partitions
NB = HW // 512   # 8 column blocks of q
CB = C // P      # 4 channel blocks
KT = HW // P     # 32 transposed k-chunks
LAG = 3          # transpose-ahead distance before mm1 consumes a k-chunk

F32 = mybir.dt.float32
F32R = mybir.dt.float32r
AF = mybir.ActivationFunctionType
ALU = mybir.AluOpType
AX = mybir.AxisListType

COPY_ENGINE = "scalar"   # engine for psum->sbuf qT copies ("scalar" or "vector")

_CACHE = {}


def _build(loop_t=None):
    nc = bacc.Bacc("TRN2", target_bir_lowering=False, debug=False)

    x_d = nc.dram_tensor("x", [C, HW], F32R, kind="ExternalInput").ap()
    at_d = nc.dram_tensor("atten", [C, C], F32, kind="ExternalInput").ap()
    g_d = nc.dram_tensor("gamma_b", [P, 1], F32, kind="ExternalInput").ap()
    out_d = nc.dram_tensor("out", [C, HW], F32, kind="ExternalOutput").ap()

    with tile.TileContext(nc) as tc:
        from contextlib import ExitStack
        with (
            tc.tile_pool(name="qp", bufs=1) as qp,
            tc.tile_pool(name="qtp", bufs=1) as qtp,
            tc.tile_pool(name="smax", bufs=2) as smax,
            tc.tile_pool(name="small", bufs=1) as small,
            tc.tile_pool(name="outp", bufs=2) as outp,
            tc.tile_pool(name="ps_sh", bufs=4, space="PSUM") as ps_sh,
            tc.tile_pool(name="ps_e", bufs=1, space="PSUM") as ps_e,
            ExitStack() as loop_ctx,
        ):
            # ---- constants
            gam = small.tile([P, 1], F32, tag="gam")
            ident = small.tile([P, P], F32, tag="ident")
            masks.make_identity(nc, ident[:])
            identr = small.tile([P, P], F32R, tag="identr")
            nc.vector.tensor_copy(identr[:], ident[:])

            att_in = [small.tile([P, C], F32, tag=f"attin{ci}", name=f"attin{ci}")
                      for ci in range(CB)]

            if loop_t is not None:
                loop_ctx.enter_context(tc.For_i(0, loop_t, 1))

            # ---- load q: q[ci][h] is (128, 2048) f32r, DMA'd in (128, 512)
            # chunks ordered nb-major so transpose k-groups unlock early
            q = [[qp.tile([P, 2048], F32R, tag=f"q{ci}_{h}", name=f"q{ci}_{h}")
                  for h in range(2)] for ci in range(CB)]
            for nb in range(NB):
                h, r = divmod(nb, 4)
                for ci in range(CB):
                    nc.sync.dma_start(
                        q[ci][h][:, r * 512:(r + 1) * 512],
                        x_d[ci * P:(ci + 1) * P, nb * 512:(nb + 1) * 512])
            for ci in range(CB):
                nc.sync.dma_start(att_in[ci][:], at_d[ci * P:(ci + 1) * P, :])
            nc.sync.dma_start(gam[:], g_d[:])

            def q_slice(ci, nb):
                """(128, 512) f32r view of channel-block ci, n-block nb."""
                h, r = divmod(nb, 4)
                return q[ci][h][:, r * 512:(r + 1) * 512]

            def q_kslice(ci, k):
                """(128, 128) f32r view for transpose chunk k."""
                h, r = divmod(k, 16)
                return q[ci][h][:, r * P:(r + 1) * P]

            # ---- interleaved: PE transposes (k) + mm1 matmuls (k - LAG)
            qt = [qtp.tile([P, C], F32R, tag=f"qt{k}", name=f"qt{k}")
                  for k in range(KT)]
            pe_ = [ps_e.tile([P, C], F32, tag=f"e{mi}", name=f"pe{mi}")
                   for mi in range(CB)]
            copy_eng = getattr(nc, COPY_ENGINE)

            def emit_transpose(k):
                ptr = ps_sh.tile([P, C], F32R, tag="sh", name=f"tr{k}")
                for ci in range(CB):
                    nc.tensor.transpose(
                        ptr[:, ci * P:(ci + 1) * P], q_kslice(ci, k), identr[:])
                if COPY_ENGINE == "scalar":
                    copy_eng.copy(qt[k][:], ptr[:])
                else:
                    copy_eng.tensor_copy(qt[k][:], ptr[:])

            def emit_mm1(k, mis):
                for mi in mis:
                    nc.tensor.matmul(
                        pe_[mi][:], qt[k][:, mi * P:(mi + 1) * P], qt[k][:],
                        start=(k == 0), stop=(k == KT - 1))

            # interleave transposes with mm1 for k < KT-TAIL; then stagger
            # the last TAIL k's per mi so the softmax latency chain for
            # mi=0 hides under the mm1 tails of mi=1..3
            TAIL = 16
            for k in range(KT + LAG):
                if k < KT:
                    emit_transpose(k)
                if k >= LAG and k - LAG < KT - TAIL:
                    emit_mm1(k - LAG, range(CB))
            for mi in range(CB):
                for k in range(KT - TAIL, KT):
                    emit_mm1(k, [mi])

            # ---- softmax chain per channel-block mi
            a2 = [smax.tile([P, C], F32R, tag=f"a2_{mi}", name=f"a2_{mi}")
                  for mi in range(CB)]
            for mi in range(CB):
                pe = pe_[mi]
                # softmax(rowmax - energy) == exp(rowmin - energy) / sum
                rmin = smax.tile([P, 1], F32, tag="rmin")
                nc.vector.tensor_reduce(rmin[:], pe[:], axis=AX.X, op=ALU.min)
                e1 = smax.tile([P, C], F32, tag="e1")
                s1 = smax.tile([P, 1], F32, tag="s1")
                nc.scalar.activation(
                    e1[:], pe[:], AF.Exp, bias=rmin[:], scale=-1.0,
                    accum_out=s1[:])
                r1 = smax.tile([P, 1], F32, tag="r1")
                nc.vector.reciprocal(r1[:], s1[:])

                z = smax.tile([P, C], F32, tag="z")
                nc.vector.scalar_tensor_tensor(
                    z[:], e1[:], r1[:], att_in[mi][:], op0=ALU.mult, op1=ALU.add)
                nm2 = smax.tile([P, 1], F32, tag="nm2")
                nc.vector.tensor_reduce(
                    nm2[:], z[:], axis=AX.X, op=ALU.max, negate=True)
                e2 = smax.tile([P, C], F32, tag="e2")
                s2 = smax.tile([P, 1], F32, tag="s2")
                nc.scalar.activation(
                    e2[:], z[:], AF.Exp, bias=nm2[:], scale=1.0, accum_out=s2[:])
                r2 = smax.tile([P, 1], F32, tag="r2")
                nc.vector.reciprocal(r2[:], s2[:])
                nc.vector.tensor_scalar_mul(a2[mi][:], e2[:], r2[:])

            # ---- mm2 + epilogue; out staged per (mj, half) -> 8 x 1MB DMAs
            # epilogue alternates between a single DVE op and an ACT-mul +
            # DVE-add pair to balance the two engines in this phase
            def emit_epilogue(ot, mj, h, r, po):
                nb = h * 4 + r
                osl = ot[:, r * 512:(r + 1) * 512]
                if nb % 2 == 0:
                    nc.vector.scalar_tensor_tensor(
                        osl, po[:], gam[:],
                        q_slice(mj, nb).bitcast(F32),
                        op0=ALU.mult, op1=ALU.add)
                else:
                    go = smax.tile([P, 512], F32, tag="go", name=f"go{mj}_{nb}")
                    nc.scalar.mul(go[:], po[:], gam[:])
                    nc.vector.tensor_add(
                        osl, go[:], q_slice(mj, nb).bitcast(F32))
                if nb % 2 == 1:
                    nc.sync.dma_start(
                        out_d[mj * P:(mj + 1) * P, (nb - 1) * 512:(nb + 1) * 512],
                        ot[:, (r - 1) * 512:(r + 1) * 512])

            # first (mj=0, h=0) quadruple: ki-major accumulation across the
            # four groups so PE has a2[ki]-independent work while the last
            # softmax chains finish
            ot0 = outp.tile([P, 2048], F32, tag="ot", name="ot0_0")
            po0 = [ps_sh.tile([P, 512], F32, tag="sh", name=f"po0_{r}")
                   for r in range(4)]
            for ki in range(CB):
                for r in range(4):
                    nc.tensor.matmul(
                        po0[r][:], a2[ki][:, 0:P], q_slice(ki, r),
                        start=(ki == 0), stop=(ki == CB - 1))
            for r in range(4):
                emit_epilogue(ot0, 0, 0, r, po0[r])

            for mj in range(CB):
                for h in range(2):
                    if mj == 0 and h == 0:
                        continue
                    ot = outp.tile([P, 2048], F32, tag="ot", name=f"ot{mj}_{h}")
                    for r in range(4):
                        nb = h * 4 + r
                        po = ps_sh.tile([P, 512], F32, tag="sh", name=f"po{mj}_{nb}")
                        for ki in range(CB):
                            nc.tensor.matmul(
                                po[:], a2[ki][:, mj * P:(mj + 1) * P],
                                q_slice(ki, nb),
                                start=(ki == 0), stop=(ki == CB - 1))
                        emit_epilogue(ot, mj, h, r, po)

    nc.compile()
    return nc


def get_nc(loop_t=None):
    key = ("nc", loop_t)
    if key not in _CACHE:
        _CACHE[key] = _build(loop_t)
    return _CACHE[key]


def make_in_maps(inputs):
    x, atten, gamma = inputs["x"], inputs["atten"], inputs["gamma"]
    gb = np.broadcast_to(np.asarray(gamma, np.float32).reshape(1, 1), (P, 1)).copy()
    return [
        {
            "x": np.ascontiguousarray(np.asarray(x[d], np.float32).reshape(C, HW)),
            "atten": np.ascontiguousarray(np.asarray(atten[d], np.float32)),
            "gamma_b": gb,
        }
        for d in range(D)
    ]


class _Executor:
    """Prebuilt sharded PJRT executable for repeat kernel() calls.

    Mirrors bass2jax.run_bass_via_pjrt's multi-core path, but the jitted
    program is built once and reused, so repeat calls only pay transfers.
    """

    def __init__(self, nc):
        import jax
        import jax.numpy as jnp
        from jax.experimental.shard_map import shard_map
        from jax.sharding import Mesh, PartitionSpec, NamedSharding
        from concourse import bass2jax

        bass2jax.install_neuronx_cc_hook()
        self._jax = jax
        pname = nc.partition_id_tensor.name if nc.partition_id_tensor else None
        in_names, out_names, out_avals = [], [], []
        for alloc in nc.m.functions[0].allocations:
            if not isinstance(alloc, mybir.MemoryLocationSet):
                continue
            name = alloc.memorylocations[0].name
            if alloc.kind == "ExternalInput":
                if name != pname:
                    in_names.append(name)
            elif alloc.kind == "ExternalOutput":
                out_names.append(name)
                out_avals.append(jax.core.ShapedArray(
                    tuple(alloc.tensor_shape), mybir.dt.np(alloc.dtype)))
        self.in_names, self.out_names, self.out_avals = \
            in_names, out_names, out_avals
        all_in = in_names + out_names + ([pname] if pname else [])

        def _body(*args):
            operands = list(args)
            if pname is not None:
                operands.append(bass2jax.partition_id_tensor())
            return tuple(bass2jax._bass_exec_p.bind(
                *operands,
                out_avals=tuple(out_avals),
                in_names=tuple(all_in),
                out_names=tuple(out_names),
                lowering_input_output_aliases=(),
                sim_require_finite=True,
                sim_require_nnan=True,
                nc=nc,
            ))

        devices = jax.devices()[:D]
        mesh = Mesh(np.asarray(devices), ("core",))
        n_params, n_outs = len(in_names), len(out_names)
        self.sharding = NamedSharding(mesh, PartitionSpec("core"))
        self.sharded = jax.jit(
            shard_map(
                _body, mesh=mesh,
                in_specs=(PartitionSpec("core"),) * (n_params + n_outs),
                out_specs=(PartitionSpec("core"),) * n_outs,
                check_rep=False,
            ),
            donate_argnums=tuple(range(n_params, n_params + n_outs)),
            keep_unused=True,
        )
        zshapes = [(D * a.shape[0], *a.shape[1:]) for a in out_avals]
        zdtypes = [a.dtype for a in out_avals]
        self.mk_zeros = jax.jit(
            lambda: tuple(jnp.zeros(s, d) for s, d in zip(zshapes, zdtypes)),
            out_shardings=tuple(self.sharding for _ in zshapes),
        )

    def run(self, in_maps):
        jax = self._jax
        din = [
            jax.device_put(
                np.concatenate([np.asarray(m[name]) for m in in_maps], axis=0),
                self.sharding)
            for name in self.in_names
        ]
        outs = self.sharded(*din, *self.mk_zeros())
        return [
            {name: np.asarray(outs[i]).reshape(D, *self.out_avals[i].shape)[d]
             for i, name in enumerate(self.out_names)}
            for d in range(D)
        ]


def _get_executor():
    if "exec" not in _CACHE:
        _CACHE["exec"] = _Executor(get_nc())
    return _CACHE["exec"]


def kernel(x: np.ndarray, atten: np.ndarray, gamma: np.ndarray) -> np.ndarray:
    assert x.shape == (D, C, 64, 64) and atten.shape == (D, C, C)
    in_maps = make_in_maps({"x": x, "atten": atten, "gamma": gamma})
    try:
        results = _get_executor().run(in_maps)
    except Exception:
        res = bass_utils.run_bass_kernel_spmd(get_nc(), in_maps, list(range(D)))
        results = res.results
    out = np.stack([results[d]["out"] for d in range(D)])
    return out.reshape(D, C, 64, 64).astype(np.float32)
